# revision 1
# baseline (speedup 1.0000x reference)
"""Trainium2 Bass kernel for nn_AdaptiveNodeClassifier — V2 architecture.

Nodes sharded across 8 cores (dest-owner edge partition). Per layer:
z' = dinv*(feat@W) per shard -> AllGather -> bf16 table [100352,128] in DRAM.
Edges are arranged dest-tile-major with a globally uniform chunk count m per
dest tile (static SPMD schedule); per 128-edge chunk, source rows are fetched
with indirect_dma_start (standard DGE dynamic DMA) and scatter-added into the
dest tile's PSUM accumulator via a one-hot (dst%128) matmul on TensorE.
Self-loop term is added from the local z' tile. Head: low/high mix, 2-layer
MLP, indexed injection, log_softmax — all per dest tile on-chip.
"""

from dataclasses import dataclass, field

import numpy as np
import ml_dtypes

import concourse.bacc as bacc
import concourse.bass as bass
import concourse.mybir as mybir
import concourse.tile as tile

F32 = mybir.dt.float32
BF16 = mybir.dt.bfloat16
I32 = mybir.dt.int32

HID = 128
C = 10
LAM = 0.8


@dataclass
class Cfg:
    ncores: int = 8
    npc: int = 12500
    m: int = 0          # chunks of 128 edges per dest tile (set by host plan)
    gform: str = "B"    # "A": one [128,m] gather per tile; "B": m [128,1] gathers
    perms: list = field(default_factory=list)  # per-core old->new local node id

    @property
    def npc_pad(self):
        return (self.npc + 127) // 128 * 128 if self.npc % 128 else self.npc + 128

    @property
    def nt(self):
        return self.npc_pad // 128

    @property
    def n(self):
        return self.ncores * self.npc


def build_host_plan(cfg: Cfg, inputs: dict) -> list[dict]:
    x = np.asarray(inputs["x"], np.float32)
    ei = np.asarray(inputs["edge_index"]).astype(np.int64)
    inj = np.asarray(inputs["inject_indices"]).astype(np.int64)
    preds = np.asarray(inputs["initial_preds"], np.float32)
    n, npc, npp = cfg.n, cfg.npc, cfg.npc_pad

    src, dst = ei[0], ei[1]
    deg = np.bincount(dst, minlength=n).astype(np.float32) + 1.0
    dinv = (1.0 / np.sqrt(deg)).astype(np.float32)

    owner = dst // npc
    # Rebalance nodes across dest tiles per core (LPT, <=128 nodes/tile)
    # to minimize the max per-tile edge count -> smaller m.
    dstl = dst % npc
    cfg.perms = []       # per core: old local id -> new local id (in [0, npc_pad))
    maxcnt = 0
    for c in range(cfg.ncores):
        degl = np.bincount(dstl[owner == c], minlength=npc)
        order = np.argsort(-degl, kind="stable")
        bin_load = np.zeros(cfg.nt, np.int64)
        bin_n = np.zeros(cfg.nt, np.int64)
        newid = np.zeros(npc, np.int64)
        for i in order:
            open_b = bin_n < 128
            b = np.flatnonzero(open_b)[np.argmin(bin_load[open_b])]
            newid[i] = b * 128 + bin_n[b]
            bin_n[b] += 1
            bin_load[b] += degl[i]
        cfg.perms.append(newid)
        maxcnt = max(maxcnt, int(bin_load.max()))
    cfg.m = (maxcnt + 127) // 128

    mask = np.zeros(n, np.float32)
    mask[np.unique(inj)] = 1.0

    # global source row in the AllGathered (permuted) table
    perm_all = np.concatenate(
        [c * npp + cfg.perms[c] for c in range(cfg.ncores)])
    src_row = perm_all[src]

    in_maps = []
    for c in range(cfg.ncores):
        sl = slice(c * npc, (c + 1) * npc)
        perm = cfg.perms[c]                      # old -> new (0..npp)
        perm_inv = np.full(npp, npc, np.int64)   # new -> old (pad -> npc)
        perm_inv[perm] = np.arange(npc)
        perm_inv = np.minimum(perm_inv, npc - 1)
        occupied = np.zeros(npp, bool)
        occupied[perm] = True
        xt = np.zeros((128, npp), np.float32)
        xt[:, occupied] = x[sl].T[:, perm_inv[occupied]]
        tmp = np.ones(npp, np.float32)
        tmp[occupied] = dinv[sl][perm_inv[occupied]]
        dv = np.ascontiguousarray(tmp.reshape(cfg.nt, 128).T)
        tmp = np.zeros(npp, np.float32)
        tmp[occupied] = mask[sl][perm_inv[occupied]]
        mk = np.ascontiguousarray(tmp.reshape(cfg.nt, 128).T)
        pr = np.zeros((npp, C), np.float32)
        pr[occupied] = preds[sl][perm_inv[occupied]]
        pt = np.ascontiguousarray(
            pr.reshape(cfg.nt, 128, C).transpose(1, 0, 2)).reshape(128, cfg.nt * C)

        # edge schedule: dest-tile-major, m*128 slots per tile
        oc = owner == c
        dnew = perm[dstl[oc]]                 # new local id of each edge dst
        rows_all = src_row[oc]
        dt_new = dnew // 128
        lo_new = (dnew % 128).astype(np.float32)
        gidx = np.zeros((128, cfg.nt * cfg.m), np.int64)
        dlo = np.full((128, cfg.nt * cfg.m), -1.0, np.float32)
        for d in range(cfg.nt):
            mmask = dt_new == d
            k = int(mmask.sum())
            pp = np.arange(k) % 128
            jj = np.arange(k) // 128
            gidx[pp, d * cfg.m + jj] = rows_all[mmask]
            dlo[pp, d * cfg.m + jj] = lo_new[mmask]
        im = {
            "xt": xt.astype(ml_dtypes.bfloat16),
            "dinv": dv,
            "maskt": mk,
            "predst": pt,
            "gidx": gidx.astype(np.int32),
            "dstlo": dlo.astype(np.float32),
            "iota": np.broadcast_to(
                np.arange(128, dtype=np.float32),
                (128, 128)).astype(ml_dtypes.bfloat16).copy(),
            "identb": np.eye(128, dtype=ml_dtypes.bfloat16),
            "identf": np.eye(128, dtype=np.float32),
            "w1": np.asarray(inputs["W1"], np.float32),
            "w2": np.asarray(inputs["W2"], np.float32),
            "wl1": np.asarray(inputs["Wl1"], np.float32),
            "wl2": np.asarray(inputs["Wl2"], np.float32),
            "wm1": np.asarray(inputs["Wm1"], np.float32),
            "wm2": np.asarray(inputs["Wm2"], np.float32),
            "b1r": np.tile(np.asarray(inputs["b1"], np.float32), (128, 1)),
            "b2r": np.tile(np.asarray(inputs["b2"], np.float32), (128, 1)),
            "bm1c": np.asarray(inputs["bm1"], np.float32).reshape(128, 1),
            "bm2c": np.asarray(inputs["bm2"], np.float32).reshape(C, 1),
        }
        in_maps.append(im)
    return in_maps


def build_graph(cfg: Cfg) -> bacc.Bacc:
    nc = bacc.Bacc("TRN2", target_bir_lowering=False, debug=False,
                   num_devices=cfg.ncores)
    npp, nt, m = cfg.npc_pad, cfg.nt, cfg.m

    xt_d = nc.dram_tensor("xt", [128, npp], BF16, kind="ExternalInput")
    dinv_d = nc.dram_tensor("dinv", [128, nt], F32, kind="ExternalInput")
    mask_d = nc.dram_tensor("maskt", [128, nt], F32, kind="ExternalInput")
    preds_d = nc.dram_tensor("predst", [128, nt * C], F32, kind="ExternalInput")
    gidx_d = nc.dram_tensor("gidx", [128, nt * m], I32, kind="ExternalInput")
    dstlo_d = nc.dram_tensor("dstlo", [128, nt * m], F32, kind="ExternalInput")
    iota_d = nc.dram_tensor("iota", [128, 128], BF16, kind="ExternalInput")
    identb_d = nc.dram_tensor("identb", [128, 128], BF16, kind="ExternalInput")
    identf_d = nc.dram_tensor("identf", [128, 128], F32, kind="ExternalInput")
    wd = {k: nc.dram_tensor(k, [128, 128], F32, kind="ExternalInput")
          for k in ["w1", "w2", "wl1", "wl2", "wm1"]}
    wm2_d = nc.dram_tensor("wm2", [128, C], F32, kind="ExternalInput")
    b1r_d = nc.dram_tensor("b1r", [128, 128], F32, kind="ExternalInput")
    b2r_d = nc.dram_tensor("b2r", [128, 128], F32, kind="ExternalInput")
    bm1c_d = nc.dram_tensor("bm1c", [128, 1], F32, kind="ExternalInput")
    bm2c_d = nc.dram_tensor("bm2c", [C, 1], F32, kind="ExternalInput")
    out_d = nc.dram_tensor("out", [npp, C], F32, kind="ExternalOutput")

    zloc = [nc.dram_tensor(f"z{i}loc", [npp, HID], BF16) for i in (1, 2)]
    ztab = [nc.dram_tensor(f"ztab{i}", [cfg.ncores * npp, HID], BF16,
                           addr_space="Shared") for i in (1, 2)]
    rg = [list(range(cfg.ncores))]

    with tile.TileContext(nc) as tc:
        with (
            tc.tile_pool(name="const", bufs=1) as const,
            tc.tile_pool(name="work", bufs=4) as work,
            tc.tile_pool(name="msg", bufs=3) as msgp,
            tc.tile_pool(name="psum", bufs=2, space="PSUM") as psum,
            tc.tile_pool(name="psumb", bufs=2, space="PSUM") as psumb,
        ):
            def load_const(dram, shape, dtype=F32):
                t = const.tile(shape, dtype, tag=dram.name, name=f"{dram.name}_sb")
                nc.sync.dma_start(t[:], dram[:])
                return t

            dinv_t = load_const(dinv_d, [128, nt])
            mask_t = load_const(mask_d, [128, nt])
            preds_t = load_const(preds_d, [128, nt * C])
            gidx_t = load_const(gidx_d, [128, nt * m], I32)
            dstlo_t = load_const(dstlo_d, [128, nt * m])
            iota_t = load_const(iota_d, [128, 128], BF16)
            identb = load_const(identb_d, [128, 128], BF16)
            identf = load_const(identf_d, [128, 128], F32)
            b1r_t = load_const(b1r_d, [128, 128])
            b2r_t = load_const(b2r_d, [128, 128])
            bm1c_t = load_const(bm1c_d, [128, 1])
            bm2c_t = load_const(bm2c_d, [C, 1])

            wb = {}
            for k in ["w1", "w2", "wl1", "wl2"]:
                wf = work.tile([128, 128], F32, tag="wload", name="wf")
                nc.sync.dma_start(wf[:], wd[k][:])
                wb[k] = const.tile([128, 128], BF16, tag=f"{k}b", name=f"{k}b")
                nc.vector.tensor_copy(wb[k][:], wf[:])
            wf = work.tile([128, 128], F32, tag="wload", name="wf1")
            nc.sync.dma_start(wf[:], wd["wm1"][:])
            wm1b = const.tile([128, 128], BF16, tag="wm1b", name="wm1b")
            nc.scalar.activation(wm1b[:], wf[:],
                                 mybir.ActivationFunctionType.Copy, scale=0.5)
            wf = work.tile([128, C], F32, tag="wm2load", name="wf2")
            nc.sync.dma_start(wf[:], wm2_d[:])
            wm2b = const.tile([128, C], BF16, tag="wm2b", name="wm2b")
            nc.vector.tensor_copy(wm2b[:], wf[:])

            xtb = const.tile([128, npp], BF16, tag="xtb", name="xtb")
            nc.sync.dma_start(xtb[:], xt_d[:])

            htb = const.tile([128, npp], BF16, tag="htb", name="htb")

            def make_table(lhsT, w_t, z_d):
                for d in range(nt):
                    ps = psum.tile([128, 128], F32, tag="ps", name="ps")
                    nc.tensor.matmul(ps[:], lhsT[:, d * 128:(d + 1) * 128], w_t[:])
                    zb = work.tile([128, 128], BF16, tag="ztile", name="zb")
                    nc.vector.tensor_scalar(
                        zb[:], ps[:], dinv_t[:, d:d + 1], None,
                        mybir.AluOpType.mult)
                    nc.sync.dma_start(z_d[d * 128:(d + 1) * 128, :], zb[:])

            def seg_psum(tab, d):
                """Gather + one-hot matmul accumulate dest tile d -> psum tile."""
                ps = psum.tile([128, 128], F32, tag="acc", name="acc")
                if cfg.gform == "A":
                    mt = msgp.tile([128, m, 128], BF16, tag="msg", name="mt")
                    nc.gpsimd.indirect_dma_start(
                        out=mt[:], out_offset=None, in_=tab[:],
                        in_offset=bass.IndirectOffsetOnAxis(
                            ap=gidx_t[:, d * m:(d + 1) * m], axis=0))
                for j in range(m):
                    col = d * m + j
                    if cfg.gform == "B":
                        mtj = msgp.tile([128, 128], BF16, tag="msg", name="mtj",
                                        bufs=12)
                        nc.gpsimd.indirect_dma_start(
                            out=mtj[:], out_offset=None, in_=tab[:],
                            in_offset=bass.IndirectOffsetOnAxis(
                                ap=gidx_t[:, col:col + 1], axis=0))
                        rhs = mtj[:]
                    else:
                        rhs = mt[:, j, :]
                    oh = work.tile([128, 128], BF16, tag="oh", name="oh", bufs=8)
                    nc.vector.tensor_scalar(
                        oh[:], iota_t[:], dstlo_t[:, col:col + 1], None,
                        mybir.AluOpType.is_equal)
                    nc.tensor.matmul(ps[:], oh[:], rhs,
                                     start=(j == 0), stop=(j == m - 1))
                return ps

            def layer(tab, z_d, lhsT_next, w_high, b_t, relu):
                """Per dest tile: low = dinv*(seg+zl)+b; out = f(0.5*(low+high))."""
                for d in range(nt):
                    dsl = slice(d * 128, (d + 1) * 128)
                    ps = seg_psum(tab, d)
                    zl = work.tile([128, 128], BF16, tag="zl", name="zl")
                    nc.sync.dma_start(zl[:], z_d[dsl, :])
                    zlf = work.tile([128, 128], F32, tag="zlf", name="zlf")
                    nc.vector.tensor_copy(zlf[:], zl[:])
                    v = work.tile([128, 128], F32, tag="v", name="v")
                    nc.vector.tensor_tensor(v[:], ps[:], zlf[:],
                                            mybir.AluOpType.add)
                    low = work.tile([128, 128], F32, tag="low", name="low")
                    nc.vector.tensor_scalar(low[:], v[:], dinv_t[:, d:d + 1],
                                            None, mybir.AluOpType.mult)
                    u = work.tile([128, 128], F32, tag="u", name="u")
                    nc.vector.tensor_tensor(u[:], low[:], b_t[:],
                                            mybir.AluOpType.add)
                    hp = psum.tile([128, 128], F32, tag="ps", name="hp")
                    nc.tensor.matmul(hp[:], lhsT_next[:, dsl], w_high[:])
                    w = work.tile([128, 128], F32, tag="w", name="w")
                    nc.vector.tensor_tensor(w[:], u[:], hp[:],
                                            mybir.AluOpType.add)
                    yield d, dsl, w

            # ---------- layer 1 ----------
            make_table(xtb, wb["w1"], zloc[0])
            nc.gpsimd.collective_compute(
                "AllGather", mybir.AluOpType.bypass,
                ins=[zloc[0][:]], outs=[ztab[0][:]], replica_groups=rg)
            for d, dsl, w in layer(ztab[0], zloc[0], xtb, wb["wl1"], b1r_t, True):
                hb = work.tile([128, 128], BF16, tag="hb", name="hb")
                nc.scalar.activation(hb[:], w[:],
                                     mybir.ActivationFunctionType.Relu, scale=0.5)
                pt = psumb.tile([128, 128], BF16, tag="ptb", name="ptb")
                nc.tensor.transpose(pt[:], hb[:], identb[:])
                nc.vector.tensor_copy(htb[:, dsl], pt[:])

            # ---------- layer 2 ----------
            make_table(htb, wb["w2"], zloc[1])
            nc.gpsimd.collective_compute(
                "AllGather", mybir.AluOpType.bypass,
                ins=[zloc[1][:]], outs=[ztab[1][:]], replica_groups=rg)
            for d, dsl, w in layer(ztab[1], zloc[1], htb, wb["wl2"], b2r_t, False):
                h2p = work.tile([128, 128], BF16, tag="h2p", name="h2p")
                nc.scalar.activation(h2p[:], w[:],
                                     mybir.ActivationFunctionType.Copy)
                pt = psumb.tile([128, 128], BF16, tag="ptb", name="ptb2")
                nc.tensor.transpose(pt[:], h2p[:], identb[:])
                h2pt = work.tile([128, 128], BF16, tag="h2pt", name="h2pt")
                nc.vector.tensor_copy(h2pt[:], pt[:])
                t1p = psum.tile([128, 128], F32, tag="hd", name="t1p", bufs=2)
                nc.tensor.matmul(t1p[:], wm1b[:], h2pt[:])
                t1t = work.tile([128, 128], BF16, tag="t1t", name="t1t")
                nc.scalar.activation(t1t[:], t1p[:],
                                     mybir.ActivationFunctionType.Relu,
                                     bias=bm1c_t[:])
                lgp = psum.tile([C, 128], F32, tag="hd", name="lgp", bufs=2)
                nc.tensor.matmul(lgp[:], wm2b[:], t1t[:])
                lgt = work.tile([C, 128], F32, tag="lgt", name="lgt")
                nc.vector.tensor_scalar(lgt[:], lgp[:], bm2c_t[:], None,
                                        mybir.AluOpType.add)
                ptl = psum.tile([128, C], F32, tag="hd", name="ptl", bufs=2)
                nc.tensor.transpose(ptl[:], lgt[:], identf[:C, :C])
                inj = work.tile([128, C], F32, tag="inj", name="inj")
                nc.vector.tensor_scalar(
                    inj[:], preds_t[:, d * C:(d + 1) * C], mask_t[:, d:d + 1],
                    LAM, mybir.AluOpType.mult, mybir.AluOpType.mult)
                lg = work.tile([128, C], F32, tag="lg", name="lg")
                nc.vector.tensor_tensor(lg[:], ptl[:], inj[:],
                                        mybir.AluOpType.add)
                mneg = work.tile([128, 1], F32, tag="mneg", name="mneg")
                nc.vector.tensor_reduce(mneg[:], lg[:], mybir.AxisListType.X,
                                        mybir.AluOpType.max, negate=True)
                e = work.tile([128, C], F32, tag="e", name="e")
                nc.scalar.activation(e[:], lg[:],
                                     mybir.ActivationFunctionType.Exp,
                                     bias=mneg[:])
                s = work.tile([128, 1], F32, tag="s", name="s")
                nc.vector.tensor_reduce(s[:], e[:], mybir.AxisListType.X,
                                        mybir.AluOpType.add)
                ls = work.tile([128, 1], F32, tag="ls", name="ls")
                nc.scalar.activation(ls[:], s[:],
                                     mybir.ActivationFunctionType.Ln)
                o = work.tile([128, C], F32, tag="o", name="o")
                nc.vector.tensor_scalar(o[:], lg[:], mneg[:], ls[:],
                                        mybir.AluOpType.add,
                                        mybir.AluOpType.subtract)
                nc.sync.dma_start(out_d[d * 128:(d + 1) * 128, :], o[:])

    nc.compile()
    return nc


def kernel(**inputs) -> np.ndarray:
    from concourse.bass_utils import run_bass_kernel_spmd

    cfg = Cfg()
    in_maps = build_host_plan(cfg, inputs)
    nc = build_graph(cfg)
    res = run_bass_kernel_spmd(nc, in_maps, core_ids=list(range(cfg.ncores)))
    return assemble(cfg, [res.results[c]["out"] for c in range(cfg.ncores)])


def assemble(cfg, outs) -> np.ndarray:
    return np.concatenate(
        [outs[c][cfg.perms[c]] for c in range(cfg.ncores)], 0)



# revision 30
# speedup vs baseline: 1.0612x; 1.0612x over previous
"""Trainium2 Bass kernel for nn_AdaptiveNodeClassifier — V4 architecture.

Nodes sharded across 8 cores (dest-owner edge partition, LPT-balanced into
128-node dest tiles). Per layer: z' = dinv*(feat@W) per shard (kept in SBUF,
one DMA to DRAM) -> AllGather -> bf16 table [100352,128] in DRAM. Source rows
are fetched with the custom dma_gather ucode (InstDMAGatherAnt): int16
indices force 4 table segments of 25088 rows; the edge schedule groups
chunks by (4-tile group, segment) so each gather instruction moves a few
thousand rows. Scatter-add into each dest tile's PSUM accumulator via one-hot
(dst%128) matmuls on TensorE; self-loop and bias fold into PSUM via
identity/ones matmuls; dest-side dinv*0.5 runs on the scalar engine. Head:
2-layer MLP, host-precomputed injection, max-free log_softmax (logits O(1))
with batched exp/sum/ln.
"""

from dataclasses import dataclass, field

import numpy as np
import ml_dtypes

import concourse.bacc as bacc
import concourse.bass as bass
import concourse.mybir as mybir
import concourse.tile as tile
from concourse import library_config

F32 = mybir.dt.float32
BF16 = mybir.dt.bfloat16
I16 = mybir.dt.int16

HID = 128
C = 10
LAM = 0.8
MIX = 0.5  # T_HET * HP_MAX
NSEG = 4
GSZ = 4    # dest tiles per schedule group


@dataclass
class Cfg:
    ncores: int = 8
    npc: int = 12500
    m: int = 0          # max chunks per dest tile (info only)
    mbufs: int = 4      # in-flight gather buffers
    nq: int = 1         # SWDGE queues to rotate gathers across (1..4)
    nocoll: int = 0     # timing-only: replace AllGather with local DMA
    reps: int = 1       # kernel body repetitions (for slope timing)
    debug: int = 0      # add intermediate-tensor outputs
    perms: list = field(default_factory=list)  # per-core old->new local id
    sched: list = field(default_factory=list)
    ncols: int = 0
    maxU: int = 0

    @property
    def npc_pad(self):
        return (self.npc + 127) // 128 * 128 if self.npc % 128 else self.npc + 128

    @property
    def nt(self):
        return self.npc_pad // 128

    @property
    def n(self):
        return self.ncores * self.npc

    @property
    def segr(self):
        return self.ncores * self.npc_pad // NSEG  # table rows per segment


def build_host_plan(cfg: Cfg, inputs: dict) -> list[dict]:
    x = np.asarray(inputs["x"], np.float32)
    ei = np.asarray(inputs["edge_index"]).astype(np.int64)
    inj = np.asarray(inputs["inject_indices"]).astype(np.int64)
    preds = np.asarray(inputs["initial_preds"], np.float32)
    n, npc, npp, nt = cfg.n, cfg.npc, cfg.npc_pad, cfg.nt

    src, dst = ei[0], ei[1]
    deg = np.bincount(dst, minlength=n).astype(np.float32) + 1.0
    dinv = (1.0 / np.sqrt(deg)).astype(np.float32)

    owner = dst // npc
    # Rebalance nodes across dest tiles per core (LPT, <=128 nodes/tile)
    # to minimize the max per-tile edge count.
    dstl = dst % npc
    cfg.perms = []
    maxcnt = 0
    for c in range(cfg.ncores):
        degl = np.bincount(dstl[owner == c], minlength=npc)
        order = np.argsort(-degl, kind="stable")
        bin_load = np.zeros(nt, np.int64)
        bin_n = np.zeros(nt, np.int64)
        newid = np.zeros(npc, np.int64)
        for i in order:
            open_b = bin_n < 128
            b = np.flatnonzero(open_b)[np.argmin(bin_load[open_b])]
            newid[i] = b * 128 + bin_n[b]
            bin_n[b] += 1
            bin_load[b] += degl[i]
        cfg.perms.append(newid)
        maxcnt = max(maxcnt, int(bin_load.max()))
    cfg.m = (maxcnt + 127) // 128

    mask = np.zeros(n, np.float32)
    mask[np.unique(inj)] = 1.0

    # Row of a source node in the AllGathered table. zloc layout is
    # [128(partition p), nt(tile d), 128(feat)], so node newid=(d*128+p) of
    # core c sits at row c*npp + p*nt + d of the [ncores*npp, 128] table.
    newid_all = np.concatenate(cfg.perms)  # indexed by global old id
    o = src // npc
    nid = newid_all[src]
    src_row = o * npp + (nid % 128) * nt + nid // 128
    seg_of = src_row // cfg.segr

    # ---- global chunk schedule (identical across cores: SPMD) ----
    cnts = np.zeros((cfg.ncores, nt, NSEG), np.int64)
    for c in range(cfg.ncores):
        oc = owner == c
        dnew = cfg.perms[c][dstl[oc]]
        np.add.at(cnts[c], (dnew // 128, seg_of[oc]), 1)
    m_ds = np.ceil(cnts / 128).astype(np.int64).max(axis=0)  # [nt, NSEG]

    UCAP = 8  # chunks per gather unit (1024 idxs = SWDGE carveout limit)
    sched = []
    col = 0
    for g0 in range(0, nt, GSZ):
        tiles = list(range(g0, min(g0 + GSZ, nt)))
        units = []
        for s in range(NSEG):
            # chunks of this (group, seg) run, split into <=UCAP units
            flat = [(d, j) for d in tiles for j in range(int(m_ds[d, s]))]
            for w0 in range(0, len(flat), UCAP):
                window = flat[w0:w0 + UCAP]
                parts = []
                for d, _ in window:
                    if parts and parts[-1][0] == d:
                        parts[-1][1] += 1
                    else:
                        parts.append([d, 1])
                units.append({"seg": s, "col0": col,
                              "parts": [tuple(p) for p in parts]})
                col += len(window)
        sched.append({"tiles": tiles, "units": units})
    cfg.sched = sched
    cfg.ncols = ncols = col
    cfg.maxU = max(sum(k for _, k in u["parts"])
                   for grp in sched for u in grp["units"])

    # first column of each (tile, seg) chunk run; runs stay contiguous even
    # when split across <=UCAP gather units
    colbase = {}
    for grp in sched:
        for u in grp["units"]:
            cb = u["col0"]
            for d, k in u["parts"]:
                colbase.setdefault((d, u["seg"]), cb)
                cb += k

    in_maps = []
    for c in range(cfg.ncores):
        sl = slice(c * npc, (c + 1) * npc)
        perm = cfg.perms[c]                      # old -> new (0..npp)
        perm_inv = np.full(npp, npc, np.int64)   # new -> old (pad -> npc)
        perm_inv[perm] = np.arange(npc)
        perm_inv = np.minimum(perm_inv, npc - 1)
        occupied = np.zeros(npp, bool)
        occupied[perm] = True
        xt = np.zeros((128, npp), np.float32)
        xt[:, occupied] = x[sl].T[:, perm_inv[occupied]]
        tmp = np.ones(npp, np.float32)
        tmp[occupied] = dinv[sl][perm_inv[occupied]]
        dv = np.ascontiguousarray(tmp.reshape(nt, 128).T)
        inj_np = np.zeros((npp, C), np.float32)
        inj_np[occupied] = (LAM * mask[sl] * preds[sl].T).T[perm_inv[occupied]]
        injt = np.ascontiguousarray(
            inj_np.reshape(nt, 128, C).transpose(1, 0, 2)).reshape(128, nt * C)

        # edge schedule fill
        oc = owner == c
        dnew = perm[dstl[oc]]
        rows_all = src_row[oc]
        segs_all = seg_of[oc]
        d_all = dnew // 128
        lo_all = (dnew % 128).astype(np.float32)
        rowsmat = np.zeros((128, ncols), np.int64)
        dlo = np.full((128, ncols), -1.0, np.float32)
        for d in range(nt):
            dm = d_all == d
            for s in range(NSEG):
                if (d, s) not in colbase:
                    continue
                msk = dm & (segs_all == s)
                k = int(msk.sum())
                cb = colbase[(d, s)]
                if k:
                    pp = np.arange(k) % 128
                    jj = np.arange(k) // 128
                    rowsmat[pp, cb + jj] = rows_all[msk] - s * cfg.segr
                    dlo[pp, cb + jj] = lo_all[msk]
        # wrapped int16 index blocks, one per gather unit
        idx16 = np.zeros((128, 8 * ncols), np.int16)
        for grp in sched:
            for u in grp["units"]:
                cb = u["col0"]
                nch = sum(k for _, k in u["parts"])
                flat = rowsmat[:, cb:cb + nch].T.ravel()     # k = cloc*128+p
                blk = flat.reshape(-1, 16).T                 # [16, N/16]
                idx16[:, 8 * cb:8 * (cb + nch)] = np.tile(blk, (8, 1))

        bf = ml_dtypes.bfloat16
        im = {
            "xt": xt.astype(bf),
            "dinv": dv,
            "dinvh": (MIX * dv).astype(np.float32),
            "injt": injt,
            "idx16": idx16,
            "dstlo": dlo.astype(np.float32),
            "iota": np.broadcast_to(
                np.arange(128, dtype=np.float32),
                (128, 128)).astype(bf).copy(),
            "identb": np.eye(128, dtype=bf),
            "identf": np.eye(128, dtype=np.float32),
            "onesb": np.ones((128, 128), bf),
            "w1": np.asarray(inputs["W1"], np.float32).astype(bf),
            "w2": np.asarray(inputs["W2"], np.float32).astype(bf),
            "wl1h": (MIX * np.asarray(inputs["Wl1"], np.float32)).astype(bf),
            "wl2h": (MIX * np.asarray(inputs["Wl2"], np.float32)).astype(bf),
            "wm1": np.asarray(inputs["Wm1"], np.float32).astype(bf),
            "wm2": np.asarray(inputs["Wm2"], np.float32).astype(bf),
            "btile1": np.broadcast_to(
                MIX * np.asarray(inputs["b1"], np.float32) / 128,
                (128, 128)).astype(bf).copy(),
            "btile2": np.broadcast_to(
                MIX * np.asarray(inputs["b2"], np.float32) / 128,
                (128, 128)).astype(bf).copy(),
            "bm1c": np.asarray(inputs["bm1"], np.float32).reshape(128, 1),
            "bm2c": np.asarray(inputs["bm2"], np.float32).reshape(C, 1),
        }
        in_maps.append(im)
    return in_maps


def build_graph(cfg: Cfg) -> bacc.Bacc:
    nc = bacc.Bacc("TRN2", target_bir_lowering=False, debug=False,
                   num_devices=cfg.ncores, num_swdge_queues=cfg.nq)
    npp, nt, ncols, maxU = cfg.npc_pad, cfg.nt, cfg.ncols, cfg.maxU
    segr = cfg.segr

    xt_d = nc.dram_tensor("xt", [128, npp], BF16, kind="ExternalInput")
    dinv_d = nc.dram_tensor("dinv", [128, nt], F32, kind="ExternalInput")
    dinvh_d = nc.dram_tensor("dinvh", [128, nt], F32, kind="ExternalInput")
    injt_d = nc.dram_tensor("injt", [128, nt * C], F32, kind="ExternalInput")
    idx16_d = nc.dram_tensor("idx16", [128, 8 * ncols], I16,
                             kind="ExternalInput")
    dstlo_d = nc.dram_tensor("dstlo", [128, ncols], F32, kind="ExternalInput")
    iota_d = nc.dram_tensor("iota", [128, 128], BF16, kind="ExternalInput")
    identb_d = nc.dram_tensor("identb", [128, 128], BF16, kind="ExternalInput")
    identf_d = nc.dram_tensor("identf", [128, 128], F32, kind="ExternalInput")
    onesb_d = nc.dram_tensor("onesb", [128, 128], BF16, kind="ExternalInput")
    wd = {k: nc.dram_tensor(k, [128, 128], BF16, kind="ExternalInput")
          for k in ["w1", "w2", "wl1h", "wl2h", "wm1", "btile1", "btile2"]}
    wm2_d = nc.dram_tensor("wm2", [128, C], BF16, kind="ExternalInput")
    bm1c_d = nc.dram_tensor("bm1c", [128, 1], F32, kind="ExternalInput")
    bm2c_d = nc.dram_tensor("bm2c", [C, 1], F32, kind="ExternalInput")
    out_d = nc.dram_tensor("out", [128, nt * C], F32, kind="ExternalOutput")
    if cfg.debug:
        dbg = {
            "dbg_zsb1": nc.dram_tensor("dbg_zsb1", [128, nt * 128], BF16,
                                       kind="ExternalOutput"),
            "dbg_mt0": nc.dram_tensor("dbg_mt0", [128, maxU * 128], BF16,
                                      kind="ExternalOutput"),
            "dbg_htb": nc.dram_tensor("dbg_htb", [128, npp], BF16,
                                      kind="ExternalOutput"),
            "dbg_lgt": nc.dram_tensor("dbg_lgt", [128, nt * C], F32,
                                      kind="ExternalOutput"),
        }

    zloc = [nc.dram_tensor(f"z{i}loc", [128, nt, 128], BF16) for i in (1, 2)]
    ztab = [nc.dram_tensor(f"ztab{i}", [cfg.ncores * npp, HID], BF16,
                           addr_space="Shared") for i in (1, 2)]
    rg = [list(range(cfg.ncores))]

    with tile.TileContext(nc) as tc:
        with (
            tc.tile_pool(name="const", bufs=1) as const,
            tc.tile_pool(name="work", bufs=4) as work,
            tc.tile_pool(name="ohp", bufs=8) as ohp,
            tc.tile_pool(name="msg", bufs=2) as msgp,
            tc.tile_pool(name="idxp", bufs=3) as idxp,
            tc.tile_pool(name="psA", bufs=1, space="PSUM") as psA,
            tc.tile_pool(name="psB", bufs=2, space="PSUM") as psB,
            tc.tile_pool(name="psT", bufs=1, space="PSUM") as psT,
            tc.tile_pool(name="psH", bufs=1, space="PSUM") as psH,
        ):
            nc.gpsimd.load_library(library_config.mlp)

            def load_const(dram, shape, dtype=F32):
                t = const.tile(shape, dtype, tag=dram.name, name=f"{dram.name}_sb")
                nc.sync.dma_start(t[:], dram[:])
                return t

            dinv_t = load_const(dinv_d, [128, nt])
            dinvh_t = load_const(dinvh_d, [128, nt])
            injt_t = load_const(injt_d, [128, nt * C])
            dstlo_t = load_const(dstlo_d, [128, ncols])
            iota_t = load_const(iota_d, [128, 128], BF16)
            identb = load_const(identb_d, [128, 128], BF16)
            identf = load_const(identf_d, [128, 128], F32)
            onesb = load_const(onesb_d, [128, 128], BF16)
            wb = {k: load_const(d, [128, 128], BF16) for k, d in wd.items()}
            wm2b = load_const(wm2_d, [128, C], BF16)
            bm1c = load_const(bm1c_d, [128, 1])
            bm2c = load_const(bm2c_d, [C, 1])
            xtb = load_const(xt_d, [128, npp], BF16)

            htb = const.tile([128, npp], BF16, tag="htb", name="htb")
            zsb = [const.tile([128, nt * 128], BF16, tag=f"zsb{i}",
                              name=f"zsb{i}") for i in (1, 2)]
            lgt = const.tile([128, nt, C], F32, tag="lgt", name="lgt")

            def emit_body():
                dbg_mt_todo = [1] if cfg.debug else []

                def make_table(lhsT, w_t, zs, z_d):
                    for d in range(nt):
                        dsl = slice(d * 128, (d + 1) * 128)
                        ps = psA.tile([128, 128], F32, tag="agg0", name="mtps")
                        nc.tensor.matmul(ps[:], lhsT[:, dsl], w_t[:])
                        nc.scalar.activation(zs[:, dsl], ps[:],
                                             mybir.ActivationFunctionType.Copy,
                                             scale=dinv_t[:, d:d + 1])
                    nc.sync.dma_start(z_d[:, :, :], zs[:])

                def layer(tab, zs, lhsT, w_high, btile, out_cb):
                    uctr = [0]
                    for grp in cfg.sched:
                        ps_of, started = {}, {}
                        for i, d in enumerate(grp["tiles"]):
                            ps_of[d] = psA.tile([128, 128], F32,
                                                tag=f"agg{i}", name=f"agg{i}")
                            started[d] = False
                        for u in grp["units"]:
                            nch = sum(k for _, k in u["parts"])
                            N = nch * 128
                            c0 = u["col0"]
                            it = idxp.tile([128, 8 * maxU], I16, tag="idx",
                                           name="idx")
                            nc.sync.dma_start(
                                it[:, :8 * nch],
                                idx16_d[:, 8 * c0:8 * (c0 + nch)])
                            mt = msgp.tile([128, maxU, 128], BF16, tag="mt",
                                           name="mt", bufs=cfg.mbufs)
                            s = u["seg"]
                            nc.gpsimd.dma_gather(
                                mt[:, :nch, :],
                                tab[s * segr:(s + 1) * segr, :],
                                it[:, :8 * nch],
                                N, N, 128,
                                queue_num=uctr[0] % cfg.nq)
                            uctr[0] += 1
                            if dbg_mt_todo:
                                nc.sync.dma_start(dbg["dbg_mt0"][:], mt[:])
                                dbg_mt_todo.pop()
                            cloc = 0
                            for d, k in u["parts"]:
                                for j in range(k):
                                    col = c0 + cloc
                                    oh = ohp.tile([128, 128], BF16, tag="oh",
                                                  name="oh")
                                    nc.vector.tensor_scalar(
                                        oh[:], iota_t[:],
                                        dstlo_t[:, col:col + 1], None,
                                        mybir.AluOpType.is_equal)
                                    nc.tensor.matmul(
                                        ps_of[d][:], oh[:], mt[:, cloc, :],
                                        start=not started[d], stop=False)
                                    started[d] = True
                                    cloc += 1
                        for d in grp["tiles"]:
                            dsl = slice(d * 128, (d + 1) * 128)
                            ps = ps_of[d]
                            nc.tensor.matmul(ps[:], identb[:], zs[:, dsl],
                                             start=not started[d], stop=True)
                            u1b = work.tile([128, 128], BF16, tag="u1b",
                                            name="u1b")
                            nc.scalar.activation(
                                u1b[:], ps[:],
                                mybir.ActivationFunctionType.Copy,
                                scale=dinvh_t[:, d:d + 1])
                            hp = psB.tile([128, 128], F32, tag="hp", name="hp")
                            nc.tensor.matmul(hp[:], lhsT[:, dsl], w_high[:],
                                             start=True, stop=False)
                            nc.tensor.matmul(hp[:], onesb[:], btile[:],
                                             start=False, stop=False)
                            nc.tensor.matmul(hp[:], identb[:], u1b[:],
                                             start=False, stop=True)
                            out_cb(d, dsl, hp)

                def l1_out(d, dsl, hp):
                    hb = work.tile([128, 128], BF16, tag="hb", name="hb")
                    nc.scalar.activation(hb[:], hp[:],
                                         mybir.ActivationFunctionType.Relu)
                    pt = psT.tile([128, 128], BF16, tag="pt", name="pt")
                    nc.tensor.transpose(pt[:], hb[:], identb[:])
                    nc.scalar.activation(htb[:, dsl], pt[:],
                                         mybir.ActivationFunctionType.Copy)

                def l2_out(d, dsl, hp):
                    h2b = work.tile([128, 128], BF16, tag="h2b", name="h2b")
                    nc.scalar.activation(h2b[:], hp[:],
                                         mybir.ActivationFunctionType.Copy)
                    pt = psT.tile([128, 128], BF16, tag="pt", name="pt2")
                    nc.tensor.transpose(pt[:], h2b[:], identb[:])
                    h2t = work.tile([128, 128], BF16, tag="h2t", name="h2t")
                    nc.scalar.activation(h2t[:], pt[:],
                                         mybir.ActivationFunctionType.Copy)
                    big = psH.tile([128, 384], F32, tag="hd", name="big")
                    t1p = big[:, 0:128]
                    nc.tensor.matmul(t1p, wb["wm1"][:], h2t[:])
                    t1t = work.tile([128, 128], BF16, tag="t1t", name="t1t")
                    nc.scalar.activation(t1t[:], t1p,
                                         mybir.ActivationFunctionType.Relu,
                                         bias=bm1c[:])
                    lgp = big[:C, 128:256]
                    nc.tensor.matmul(lgp, wm2b[:], t1t[:])
                    lgs = work.tile([C, 128], F32, tag="lgs", name="lgs")
                    nc.vector.tensor_scalar(lgs[:], lgp, bm2c[:], None,
                                            mybir.AluOpType.add)
                    ptl = big[:, 256:256 + C]
                    nc.tensor.transpose(ptl, lgs[:], identf[:C, :C])
                    nc.vector.tensor_tensor(lgt[:, d, :], ptl,
                                            injt_t[:, d * C:(d + 1) * C],
                                            mybir.AluOpType.add)

                def allgather(i):
                    if cfg.nocoll:
                        nc.sync.dma_start(ztab[i][:npp, :], zloc[i][:, :, :])
                    else:
                        nc.gpsimd.collective_compute(
                            "AllGather", mybir.AluOpType.bypass,
                            ins=[zloc[i][:]], outs=[ztab[i][:]],
                            replica_groups=rg)

                make_table(xtb, wb["w1"], zsb[0], zloc[0])
                allgather(0)
                if cfg.debug:
                    nc.sync.dma_start(dbg["dbg_zsb1"][:], zsb[0][:])
                layer(ztab[0], zsb[0], xtb, wb["wl1h"], wb["btile1"], l1_out)
                if cfg.debug:
                    nc.sync.dma_start(dbg["dbg_htb"][:], htb[:])

                make_table(htb, wb["w2"], zsb[1], zloc[1])
                allgather(1)
                layer(ztab[1], zsb[1], htb, wb["wl2h"], wb["btile2"], l2_out)

                if cfg.debug:
                    nc.sync.dma_start(dbg["dbg_lgt"][:], lgt[:])
                # ---- batched log_softmax tail (no max subtraction: |logit|
                # is O(1) and exp is exact in f32 up to ~88) ----
                e = work.tile([128, nt, C], F32, tag="e", name="e")
                nc.scalar.activation(e[:], lgt[:],
                                     mybir.ActivationFunctionType.Exp)
                s = work.tile([128, nt], F32, tag="s", name="s")
                nc.vector.tensor_reduce(s[:], e[:], mybir.AxisListType.X,
                                        mybir.AluOpType.add)
                ls = work.tile([128, nt], F32, tag="ls", name="ls")
                nc.scalar.activation(ls[:], s[:],
                                     mybir.ActivationFunctionType.Ln)
                ot = work.tile([128, nt, C], F32, tag="ot", name="ot")
                for d in range(nt):
                    nc.vector.tensor_scalar(ot[:, d, :], lgt[:, d, :],
                                            ls[:, d:d + 1], None,
                                            mybir.AluOpType.subtract)
                nc.sync.dma_start(out_d[:], ot[:])

            for _ in range(cfg.reps):
                emit_body()

    nc.compile()
    return nc


def kernel(**inputs) -> np.ndarray:
    from concourse.bass_utils import run_bass_kernel_spmd

    cfg = Cfg()
    in_maps = build_host_plan(cfg, inputs)
    nc = build_graph(cfg)
    res = run_bass_kernel_spmd(nc, in_maps, core_ids=list(range(cfg.ncores)))
    return assemble(cfg, [res.results[c]["out"] for c in range(cfg.ncores)])


def assemble(cfg, outs) -> np.ndarray:
    nt = cfg.nt
    full = []
    for c in range(cfg.ncores):
        o = np.asarray(outs[c], np.float32).reshape(128, nt, C)
        perm = cfg.perms[c]
        full.append(o[perm % 128, perm // 128, :])
    return np.concatenate(full, 0)


# revision 41
# speedup vs baseline: 1.1829x; 1.1146x over previous
"""Trainium2 Bass kernel for nn_AdaptiveNodeClassifier — V4 architecture.

Nodes sharded across 8 cores (dest-owner edge partition, LPT-balanced into
128-node dest tiles). Per layer: z' = dinv*(feat@W) per shard (kept in SBUF,
one DMA to DRAM) -> AllGather -> bf16 table [100352,128] in DRAM. Source rows
are fetched with the custom dma_gather ucode (InstDMAGatherAnt): int16
indices force 4 table segments of 25088 rows; the edge schedule groups
chunks by (4-tile group, segment) so each gather instruction moves a few
thousand rows. Scatter-add into each dest tile's PSUM accumulator via one-hot
(dst%128) matmuls on TensorE; self-loop and bias fold into PSUM via
identity/ones matmuls; dest-side dinv*0.5 runs on the scalar engine. Head:
2-layer MLP, host-precomputed injection, max-free log_softmax (logits O(1))
with batched exp/sum/ln.
"""

from dataclasses import dataclass, field

import numpy as np
import ml_dtypes

import concourse.bacc as bacc
import concourse.bass as bass
import concourse.mybir as mybir
import concourse.tile as tile
from concourse import library_config

F32 = mybir.dt.float32
BF16 = mybir.dt.bfloat16
I16 = mybir.dt.int16

HID = 128
C = 10
LAM = 0.8
MIX = 0.5  # T_HET * HP_MAX
NSEG = 4
GSZ = 4    # dest tiles per schedule group


@dataclass
class Cfg:
    ncores: int = 8
    npc: int = 12500
    m: int = 0          # max chunks per dest tile (info only)
    mbufs: int = 6      # in-flight gather buffers
    nq: int = 1         # SWDGE queues to rotate gathers across (1..4)
    nocoll: int = 0     # timing-only: replace AllGather with local DMA
    nogather: int = 0   # timing-only: skip gather DMAs (stale SBUF reads)
    idxzero: int = 0    # timing-only: gather row 0 everywhere (page-local)
    nooh: int = 0       # timing-only: build one-hot once, reuse
    reps: int = 1       # kernel body repetitions (for slope timing)
    debug: int = 0      # add intermediate-tensor outputs
    sim: int = 0        # debug_cmp: use MultiCoreSim instead of HW
    perms: list = field(default_factory=list)  # per-core old->new local id
    sched: list = field(default_factory=list)
    ncols: int = 0
    maxU: int = 0

    @property
    def npc_pad(self):
        return (self.npc + 127) // 128 * 128 if self.npc % 128 else self.npc + 128

    @property
    def nt(self):
        return self.npc_pad // 128

    @property
    def n(self):
        return self.ncores * self.npc

    @property
    def segr(self):
        return self.ncores * self.npc_pad // NSEG  # table rows per segment


def build_host_plan(cfg: Cfg, inputs: dict) -> list[dict]:
    x = np.asarray(inputs["x"], np.float32)
    ei = np.asarray(inputs["edge_index"]).astype(np.int64)
    inj = np.asarray(inputs["inject_indices"]).astype(np.int64)
    preds = np.asarray(inputs["initial_preds"], np.float32)
    n, npc, npp, nt = cfg.n, cfg.npc, cfg.npc_pad, cfg.nt

    src, dst = ei[0], ei[1]
    deg = np.bincount(dst, minlength=n).astype(np.float32) + 1.0
    dinv = (1.0 / np.sqrt(deg)).astype(np.float32)

    owner = dst // npc
    # Rebalance nodes across dest tiles per core (LPT, <=128 nodes/tile)
    # to minimize the max per-tile edge count.
    dstl = dst % npc
    cfg.perms = []
    maxcnt = 0
    for c in range(cfg.ncores):
        degl = np.bincount(dstl[owner == c], minlength=npc)
        order = np.argsort(-degl, kind="stable")
        bin_load = np.zeros(nt, np.int64)
        bin_n = np.zeros(nt, np.int64)
        newid = np.zeros(npc, np.int64)
        for i in order:
            open_b = bin_n < 128
            b = np.flatnonzero(open_b)[np.argmin(bin_load[open_b])]
            newid[i] = b * 128 + bin_n[b]
            bin_n[b] += 1
            bin_load[b] += degl[i]
        cfg.perms.append(newid)
        maxcnt = max(maxcnt, int(bin_load.max()))
    cfg.m = (maxcnt + 127) // 128

    mask = np.zeros(n, np.float32)
    mask[np.unique(inj)] = 1.0

    # Row of a source node in the AllGathered table. zloc layout is
    # [128(partition p), nt(tile d), 128(feat)], so node newid=(d*128+p) of
    # core c sits at row c*npp + p*nt + d of the [ncores*npp, 128] table.
    newid_all = np.concatenate(cfg.perms)  # indexed by global old id
    o = src // npc
    nid = newid_all[src]
    src_row = o * npp + (nid % 128) * nt + nid // 128
    seg_of = src_row // cfg.segr

    # ---- global chunk schedule (identical across cores: SPMD) ----
    cnts = np.zeros((cfg.ncores, nt, NSEG), np.int64)
    for c in range(cfg.ncores):
        oc = owner == c
        dnew = cfg.perms[c][dstl[oc]]
        np.add.at(cnts[c], (dnew // 128, seg_of[oc]), 1)
    m_ds = np.ceil(cnts / 128).astype(np.int64).max(axis=0)  # [nt, NSEG]

    UCAP = 8  # chunks per gather unit (1024 idxs = SWDGE carveout limit)
    # One gather unit per (tile, seg) run piece, so a unit's pad slots are
    # k-trailing and can be dropped from the transfer by shrinking num_idxs
    # to the max-over-cores real edge count of the piece.
    sched = []
    col = 0
    for g0 in range(0, nt, GSZ):
        tiles = list(range(g0, min(g0 + GSZ, nt)))
        units = []
        for s in range(NSEG):
            for d in tiles:
                mds = int(m_ds[d, s])
                cmax = int(cnts[:, d, s].max())
                for j0 in range(0, mds, UCAP):
                    kk = min(UCAP, mds - j0)
                    nreal = min(max(cmax - j0 * 128, 1), kk * 128)
                    nidx = (nreal + 15) // 16 * 16
                    units.append({"seg": s, "col0": col, "parts": [(d, kk)],
                                  "nidx": nidx})
                    col += kk
        sched.append({"tiles": tiles, "units": units})
    cfg.sched = sched
    cfg.ncols = ncols = col
    cfg.maxU = max(sum(k for _, k in u["parts"])
                   for grp in sched for u in grp["units"])

    # first column of each (tile, seg) chunk run; runs stay contiguous even
    # when split across <=UCAP gather units
    colbase = {}
    for grp in sched:
        for u in grp["units"]:
            cb = u["col0"]
            for d, k in u["parts"]:
                colbase.setdefault((d, u["seg"]), cb)
                cb += k

    in_maps = []
    for c in range(cfg.ncores):
        sl = slice(c * npc, (c + 1) * npc)
        perm = cfg.perms[c]                      # old -> new (0..npp)
        perm_inv = np.full(npp, npc, np.int64)   # new -> old (pad -> npc)
        perm_inv[perm] = np.arange(npc)
        perm_inv = np.minimum(perm_inv, npc - 1)
        occupied = np.zeros(npp, bool)
        occupied[perm] = True
        xt = np.zeros((128, npp), np.float32)
        xt[:, occupied] = x[sl].T[:, perm_inv[occupied]]
        tmp = np.ones(npp, np.float32)
        tmp[occupied] = dinv[sl][perm_inv[occupied]]
        dv = np.ascontiguousarray(tmp.reshape(nt, 128).T)
        inj_np = np.zeros((npp, C), np.float32)
        inj_np[occupied] = (LAM * mask[sl] * preds[sl].T).T[perm_inv[occupied]]
        injt = np.ascontiguousarray(
            inj_np.reshape(nt, 128, C).transpose(1, 0, 2)).reshape(128, nt * C)

        # edge schedule fill
        oc = owner == c
        dnew = perm[dstl[oc]]
        rows_all = src_row[oc]
        segs_all = seg_of[oc]
        d_all = dnew // 128
        lo_all = (dnew % 128).astype(np.float32)
        rowsmat = np.zeros((128, ncols), np.int64)
        dlo = np.full((128, ncols), -1.0, np.float32)
        for d in range(nt):
            dm = d_all == d
            for s in range(NSEG):
                if (d, s) not in colbase:
                    continue
                msk = dm & (segs_all == s)
                k = int(msk.sum())
                cb = colbase[(d, s)]
                if k:
                    pp = np.arange(k) % 128
                    jj = np.arange(k) // 128
                    rowsmat[pp, cb + jj] = rows_all[msk] - s * cfg.segr
                    dlo[pp, cb + jj] = lo_all[msk]
        if cfg.idxzero:
            rowsmat[:] = 0
        # wrapped int16 index blocks, one per gather unit
        idx16 = np.zeros((128, 8 * ncols), np.int16)
        for grp in sched:
            for u in grp["units"]:
                cb = u["col0"]
                nch = sum(k for _, k in u["parts"])
                flat = rowsmat[:, cb:cb + nch].T.ravel()     # k = cloc*128+p
                blk = flat.reshape(-1, 16).T                 # [16, N/16]
                idx16[:, 8 * cb:8 * (cb + nch)] = np.tile(blk, (8, 1))

        bf = ml_dtypes.bfloat16
        im = {
            "xt": xt.astype(bf),
            "dinv": dv,
            "dinvh": (MIX * dv).astype(np.float32),
            "injt": injt,
            "idx16": idx16,
            "dstlo": dlo.astype(np.float32),
            "iota": np.broadcast_to(
                np.arange(128, dtype=np.float32),
                (128, 128)).astype(bf).copy(),
            "identb": np.eye(128, dtype=bf),
            "identf": np.eye(128, dtype=np.float32),
            "onesb": np.ones((128, 128), bf),
            "w1": np.asarray(inputs["W1"], np.float32).astype(bf),
            "w2": np.asarray(inputs["W2"], np.float32).astype(bf),
            "wl1h": (MIX * np.asarray(inputs["Wl1"], np.float32)).astype(bf),
            "wl2h": (MIX * np.asarray(inputs["Wl2"], np.float32)).astype(bf),
            "wm1": np.asarray(inputs["Wm1"], np.float32).astype(bf),
            "wm2": np.asarray(inputs["Wm2"], np.float32).astype(bf),
            "btile1": np.broadcast_to(
                MIX * np.asarray(inputs["b1"], np.float32) / 128,
                (128, 128)).astype(bf).copy(),
            "btile2": np.broadcast_to(
                MIX * np.asarray(inputs["b2"], np.float32) / 128,
                (128, 128)).astype(bf).copy(),
            "bm1c": np.asarray(inputs["bm1"], np.float32).reshape(128, 1),
            "bm2c": np.asarray(inputs["bm2"], np.float32).reshape(C, 1),
        }
        in_maps.append(im)
    return in_maps


def build_graph(cfg: Cfg) -> bacc.Bacc:
    nc = bacc.Bacc("TRN2", target_bir_lowering=False, debug=False,
                   num_devices=cfg.ncores, num_swdge_queues=cfg.nq)
    npp, nt, ncols, maxU = cfg.npc_pad, cfg.nt, cfg.ncols, cfg.maxU
    segr = cfg.segr

    xt_d = nc.dram_tensor("xt", [128, npp], BF16, kind="ExternalInput")
    dinv_d = nc.dram_tensor("dinv", [128, nt], F32, kind="ExternalInput")
    dinvh_d = nc.dram_tensor("dinvh", [128, nt], F32, kind="ExternalInput")
    injt_d = nc.dram_tensor("injt", [128, nt * C], F32, kind="ExternalInput")
    idx16_d = nc.dram_tensor("idx16", [128, 8 * ncols], I16,
                             kind="ExternalInput")
    dstlo_d = nc.dram_tensor("dstlo", [128, ncols], F32, kind="ExternalInput")
    iota_d = nc.dram_tensor("iota", [128, 128], BF16, kind="ExternalInput")
    identb_d = nc.dram_tensor("identb", [128, 128], BF16, kind="ExternalInput")
    identf_d = nc.dram_tensor("identf", [128, 128], F32, kind="ExternalInput")
    onesb_d = nc.dram_tensor("onesb", [128, 128], BF16, kind="ExternalInput")
    wd = {k: nc.dram_tensor(k, [128, 128], BF16, kind="ExternalInput")
          for k in ["w1", "w2", "wl1h", "wl2h", "wm1", "btile1", "btile2"]}
    wm2_d = nc.dram_tensor("wm2", [128, C], BF16, kind="ExternalInput")
    bm1c_d = nc.dram_tensor("bm1c", [128, 1], F32, kind="ExternalInput")
    bm2c_d = nc.dram_tensor("bm2c", [C, 1], F32, kind="ExternalInput")
    out_d = nc.dram_tensor("out", [128, nt * C], F32, kind="ExternalOutput")
    if cfg.debug:
        dbg = {
            "dbg_zsb1": nc.dram_tensor("dbg_zsb1", [128, nt * 128], BF16,
                                       kind="ExternalOutput"),
            "dbg_mt0": nc.dram_tensor("dbg_mt0", [128, maxU * 128], BF16,
                                      kind="ExternalOutput"),
            "dbg_htb": nc.dram_tensor("dbg_htb", [128, npp], BF16,
                                      kind="ExternalOutput"),
            "dbg_lgt": nc.dram_tensor("dbg_lgt", [128, nt * C], F32,
                                      kind="ExternalOutput"),
        }

    zloc = [nc.dram_tensor(f"z{i}loc", [128, nt, 128], BF16) for i in (1, 2)]
    ztab = [nc.dram_tensor(f"ztab{i}", [cfg.ncores * npp, HID], BF16,
                           addr_space="Shared") for i in (1, 2)]
    rg = [list(range(cfg.ncores))]

    with tile.TileContext(nc) as tc:
        with (
            tc.tile_pool(name="const", bufs=1) as const,
            tc.tile_pool(name="work", bufs=4) as work,
            tc.tile_pool(name="ohp", bufs=8) as ohp,
            tc.tile_pool(name="msg", bufs=2) as msgp,
            tc.tile_pool(name="idxp", bufs=3) as idxp,
            tc.tile_pool(name="psA", bufs=1, space="PSUM") as psA,
            tc.tile_pool(name="psB", bufs=2, space="PSUM") as psB,
            tc.tile_pool(name="psT", bufs=1, space="PSUM") as psT,
            tc.tile_pool(name="psH", bufs=1, space="PSUM") as psH,
        ):
            nc.gpsimd.load_library(library_config.mlp)

            def load_const(dram, shape, dtype=F32):
                t = const.tile(shape, dtype, tag=dram.name, name=f"{dram.name}_sb")
                nc.sync.dma_start(t[:], dram[:])
                return t

            dinv_t = load_const(dinv_d, [128, nt])
            dinvh_t = load_const(dinvh_d, [128, nt])
            injt_t = load_const(injt_d, [128, nt * C])
            dstlo_t = load_const(dstlo_d, [128, ncols])
            iota_t = load_const(iota_d, [128, 128], BF16)
            identb = load_const(identb_d, [128, 128], BF16)
            identf = load_const(identf_d, [128, 128], F32)
            onesb = load_const(onesb_d, [128, 128], BF16)
            wb = {k: load_const(d, [128, 128], BF16) for k, d in wd.items()}
            wm2b = load_const(wm2_d, [128, C], BF16)
            bm1c = load_const(bm1c_d, [128, 1])
            bm2c = load_const(bm2c_d, [C, 1])
            xtb = load_const(xt_d, [128, npp], BF16)

            htb = const.tile([128, npp], BF16, tag="htb", name="htb")
            zsb = [const.tile([128, nt * 128], BF16, tag=f"zsb{i}",
                              name=f"zsb{i}") for i in (1, 2)]
            lgt = const.tile([128, nt, C], F32, tag="lgt", name="lgt")

            def emit_body():
                dbg_mt_todo = [1] if cfg.debug else []

                def make_table(lhsT, w_t, zs, z_d):
                    for d in range(nt):
                        dsl = slice(d * 128, (d + 1) * 128)
                        ps = psA.tile([128, 128], F32, tag="agg0", name="mtps")
                        nc.tensor.matmul(ps[:], lhsT[:, dsl], w_t[:])
                        nc.scalar.activation(zs[:, dsl], ps[:],
                                             mybir.ActivationFunctionType.Copy,
                                             scale=dinv_t[:, d:d + 1])
                    nc.sync.dma_start(z_d[:, :, :], zs[:])

                maxG = max(sum(sum(k for _, k in u["parts"])
                               for u in grp["units"]) for grp in cfg.sched)

                def layer(tab, zs, lhsT, w_high, btile, out_cb):
                    uctr = [0]
                    for grp in cfg.sched:
                        gc0 = grp["units"][0]["col0"]
                        gcn = sum(sum(k for _, k in u["parts"])
                                  for u in grp["units"])
                        itg = idxp.tile([128, 8 * maxG], I16,
                                        tag="idx", name="idx")
                        nc.sync.dma_start(
                            itg[:, :8 * gcn],
                            idx16_d[:, 8 * gc0:8 * (gc0 + gcn)])
                        ps_of, started = {}, {}
                        for i, d in enumerate(grp["tiles"]):
                            ps_of[d] = psA.tile([128, 128], F32,
                                                tag=f"agg{i}", name=f"agg{i}")
                            started[d] = False
                        for u in grp["units"]:
                            nch = sum(k for _, k in u["parts"])
                            N = nch * 128
                            c0 = u["col0"]
                            mt = msgp.tile([128, maxU, 128], BF16, tag="mt",
                                           name="mt", bufs=cfg.mbufs)
                            if not cfg.nogather:
                                s = u["seg"]
                                co = 8 * (c0 - gc0)
                                nidx = u["nidx"]
                                ncw = (nidx + 15) // 16
                                nc.gpsimd.dma_gather(
                                    mt[:, :nch, :],
                                    tab[s * segr:(s + 1) * segr, :],
                                    itg[:, co:co + ncw],
                                    nidx, nidx, 128,
                                    queue_num=uctr[0] % cfg.nq)
                                uctr[0] += 1
                            if dbg_mt_todo:
                                nc.sync.dma_start(dbg["dbg_mt0"][:], mt[:])
                                dbg_mt_todo.pop()
                            cloc = 0
                            for d, k in u["parts"]:
                                for j in range(k):
                                    col = c0 + cloc
                                    # only lanes < nidx of the unit were
                                    # gathered; never touch unwritten SBUF
                                    # (NaN) in the matmul
                                    Kc = min(128, u["nidx"] - cloc * 128)
                                    if cfg.nooh:
                                        if not hasattr(layer, "_oh1"):
                                            layer._oh1 = ohp.tile(
                                                [128, 128], BF16, tag="oh1",
                                                name="oh1", bufs=1)
                                            nc.vector.tensor_scalar(
                                                layer._oh1[:], iota_t[:],
                                                dstlo_t[:, 0:1], None,
                                                mybir.AluOpType.is_equal)
                                        oh = layer._oh1
                                    else:
                                        oh = ohp.tile([128, 128], BF16,
                                                      tag="oh", name="oh")
                                        nc.vector.tensor_scalar(
                                            oh[:], iota_t[:],
                                            dstlo_t[:, col:col + 1], None,
                                            mybir.AluOpType.is_equal)
                                    nc.tensor.matmul(
                                        ps_of[d][:], oh[:Kc, :],
                                        mt[:Kc, cloc, :],
                                        start=not started[d], stop=False)
                                    started[d] = True
                                    cloc += 1
                        for d in grp["tiles"]:
                            dsl = slice(d * 128, (d + 1) * 128)
                            ps = ps_of[d]
                            nc.tensor.matmul(ps[:], identb[:], zs[:, dsl],
                                             start=not started[d], stop=True)
                            u1b = work.tile([128, 128], BF16, tag="u1b",
                                            name="u1b")
                            nc.scalar.activation(
                                u1b[:], ps[:],
                                mybir.ActivationFunctionType.Copy,
                                scale=dinvh_t[:, d:d + 1])
                            hp = psB.tile([128, 128], F32, tag="hp", name="hp")
                            nc.tensor.matmul(hp[:], lhsT[:, dsl], w_high[:],
                                             start=True, stop=False)
                            nc.tensor.matmul(hp[:], onesb[:], btile[:],
                                             start=False, stop=False)
                            nc.tensor.matmul(hp[:], identb[:], u1b[:],
                                             start=False, stop=True)
                            out_cb(d, dsl, hp)

                def l1_out(d, dsl, hp):
                    hb = work.tile([128, 128], BF16, tag="hb", name="hb")
                    nc.scalar.activation(hb[:], hp[:],
                                         mybir.ActivationFunctionType.Relu)
                    pt = psT.tile([128, 128], BF16, tag="pt", name="pt")
                    nc.tensor.transpose(pt[:], hb[:], identb[:])
                    nc.scalar.activation(htb[:, dsl], pt[:],
                                         mybir.ActivationFunctionType.Copy)

                def l2_out(d, dsl, hp):
                    h2b = work.tile([128, 128], BF16, tag="h2b", name="h2b")
                    nc.scalar.activation(h2b[:], hp[:],
                                         mybir.ActivationFunctionType.Copy)
                    pt = psT.tile([128, 128], BF16, tag="pt", name="pt2")
                    nc.tensor.transpose(pt[:], h2b[:], identb[:])
                    h2t = work.tile([128, 128], BF16, tag="h2t", name="h2t")
                    nc.scalar.activation(h2t[:], pt[:],
                                         mybir.ActivationFunctionType.Copy)
                    big = psH.tile([128, 384], F32, tag="hd", name="big")
                    t1p = big[:, 0:128]
                    nc.tensor.matmul(t1p, wb["wm1"][:], h2t[:])
                    t1t = work.tile([128, 128], BF16, tag="t1t", name="t1t")
                    nc.scalar.activation(t1t[:], t1p,
                                         mybir.ActivationFunctionType.Relu,
                                         bias=bm1c[:])
                    lgp = big[:C, 128:256]
                    nc.tensor.matmul(lgp, wm2b[:], t1t[:])
                    lgs = work.tile([C, 128], F32, tag="lgs", name="lgs")
                    nc.vector.tensor_scalar(lgs[:], lgp, bm2c[:], None,
                                            mybir.AluOpType.add)
                    ptl = big[:, 256:256 + C]
                    nc.tensor.transpose(ptl, lgs[:], identf[:C, :C])
                    nc.vector.tensor_tensor(lgt[:, d, :], ptl,
                                            injt_t[:, d * C:(d + 1) * C],
                                            mybir.AluOpType.add)

                def allgather(i):
                    if cfg.nocoll:
                        nc.sync.dma_start(ztab[i][:npp, :], zloc[i][:, :, :])
                    else:
                        nc.gpsimd.collective_compute(
                            "AllGather", mybir.AluOpType.bypass,
                            ins=[zloc[i][:]], outs=[ztab[i][:]],
                            replica_groups=rg)

                make_table(xtb, wb["w1"], zsb[0], zloc[0])
                allgather(0)
                if cfg.debug:
                    nc.sync.dma_start(dbg["dbg_zsb1"][:], zsb[0][:])
                layer(ztab[0], zsb[0], xtb, wb["wl1h"], wb["btile1"], l1_out)
                if cfg.debug:
                    nc.sync.dma_start(dbg["dbg_htb"][:], htb[:])

                make_table(htb, wb["w2"], zsb[1], zloc[1])
                allgather(1)
                layer(ztab[1], zsb[1], htb, wb["wl2h"], wb["btile2"], l2_out)

                if cfg.debug:
                    nc.sync.dma_start(dbg["dbg_lgt"][:], lgt[:])
                # ---- batched log_softmax tail (no max subtraction: |logit|
                # is O(1) and exp is exact in f32 up to ~88) ----
                e = work.tile([128, nt, C], F32, tag="e", name="e")
                nc.scalar.activation(e[:], lgt[:],
                                     mybir.ActivationFunctionType.Exp)
                s = work.tile([128, nt], F32, tag="s", name="s")
                nc.vector.tensor_reduce(s[:], e[:], mybir.AxisListType.X,
                                        mybir.AluOpType.add)
                ls = work.tile([128, nt], F32, tag="ls", name="ls")
                nc.scalar.activation(ls[:], s[:],
                                     mybir.ActivationFunctionType.Ln)
                ot = work.tile([128, nt, C], F32, tag="ot", name="ot")
                for d in range(nt):
                    nc.vector.tensor_scalar(ot[:, d, :], lgt[:, d, :],
                                            ls[:, d:d + 1], None,
                                            mybir.AluOpType.subtract)
                nc.sync.dma_start(out_d[:], ot[:])

            for _ in range(cfg.reps):
                emit_body()

    nc.compile()
    return nc


def kernel(**inputs) -> np.ndarray:
    from concourse.bass_utils import run_bass_kernel_spmd

    cfg = Cfg()
    in_maps = build_host_plan(cfg, inputs)
    nc = build_graph(cfg)
    res = run_bass_kernel_spmd(nc, in_maps, core_ids=list(range(cfg.ncores)))
    return assemble(cfg, [res.results[c]["out"] for c in range(cfg.ncores)])


def assemble(cfg, outs) -> np.ndarray:
    nt = cfg.nt
    full = []
    for c in range(cfg.ncores):
        o = np.asarray(outs[c], np.float32).reshape(128, nt, C)
        perm = cfg.perms[c]
        full.append(o[perm % 128, perm // 128, :])
    return np.concatenate(full, 0)
